# revision 1
# baseline (speedup 1.0000x reference)
"""QRNN forget-mult recurrence h_t = i_t*z_t + f_t*h_{t-1} on 8 NeuronCores.

Sharding: batch dim B=32 split 4-per-core (data parallel). Per core the
[T=4096, B=4, H=256] slice is viewed as [T, C=1024] channels. The recurrence
runs on the DVE TensorTensorScan instruction, which scans along the free
dimension — so f and iz are PE-transposed into [channel, time] layout in
512-step chunks, scanned with a carried initial value, and transposed back.
"""

import numpy as np

T = 4096
B = 32
H = 256
NCORES = 8
BS = B // NCORES          # batches per core
C = BS * H                # channels per core
P = 128                   # partitions
NG = C // P               # channel groups per core
SC = 512                  # timesteps per scan chunk (one PSUM bank of f32)
NS = T // SC
NJ = SC // P              # 128-row transpose tiles per chunk

_CACHE = {}


def _build_nc():
    import concourse.tile as tile
    from concourse import bacc, mybir
    from concourse.masks import make_identity

    f32 = mybir.dt.float32
    mult = mybir.AluOpType.mult
    add = mybir.AluOpType.add

    nc = bacc.Bacc("TRN2", target_bir_lowering=False, debug=False)
    f_d = nc.dram_tensor("f", [T, C], f32, kind="ExternalInput")
    i_d = nc.dram_tensor("i", [T, C], f32, kind="ExternalInput")
    z_d = nc.dram_tensor("z", [T, C], f32, kind="ExternalInput")
    h0_d = nc.dram_tensor("h0", [C, 1], f32, kind="ExternalInput")
    h_d = nc.dram_tensor("h", [T, C], f32, kind="ExternalOutput")

    with tile.TileContext(nc) as tc:
        with (
            tc.tile_pool(name="const", bufs=1) as constp,
            tc.tile_pool(name="ins", bufs=6) as insp,
            tc.tile_pool(name="izp", bufs=6) as izp,
            tc.tile_pool(name="izts", bufs=4) as iztsp,
            tc.tile_pool(name="hts", bufs=6) as htp,
            tc.tile_pool(name="asm", bufs=2) as asmp,
            tc.tile_pool(name="psA", bufs=2, space="PSUM") as psA,
            tc.tile_pool(name="psB", bufs=2, space="PSUM") as psB,
            tc.tile_pool(name="psC", bufs=2, space="PSUM") as psC,
        ):
            ident = constp.tile([P, P], f32)
            make_identity(nc, ident[:])

            carries = []
            for g in range(NG):
                cg = constp.tile([P, 1], f32, tag=f"carry{g}")
                nc.sync.dma_start(cg[:], h0_d[g * P:(g + 1) * P, :])
                carries.append(cg)

            for s in range(NS):
                fts, izts = [], []
                for j in range(NJ):
                    t0 = s * SC + j * P
                    ft = insp.tile([P, C], f32, tag="f")
                    nc.sync.dma_start(ft[:], f_d[t0:t0 + P, :])
                    it = insp.tile([P, C], f32, tag="i")
                    nc.sync.dma_start(it[:], i_d[t0:t0 + P, :])
                    zt = insp.tile([P, C], f32, tag="z")
                    nc.sync.dma_start(zt[:], z_d[t0:t0 + P, :])
                    izt = izp.tile([P, C], f32, tag="iz")
                    nc.vector.tensor_mul(izt[:], it[:], zt[:])
                    fts.append(ft)
                    izts.append(izt)

                hasm = asmp.tile([P, NJ, C], f32, tag="hasm")
                for g in range(NG):
                    gs = slice(g * P, (g + 1) * P)
                    fT = psA.tile([P, SC], f32, tag="fT")
                    izT = psB.tile([P, SC], f32, tag="izT")
                    for j in range(NJ):
                        nc.tensor.transpose(fT[:, j * P:(j + 1) * P],
                                            fts[j][:, gs], ident[:])
                        nc.tensor.transpose(izT[:, j * P:(j + 1) * P],
                                            izts[j][:, gs], ident[:])
                    # scan operands cannot both live in PSUM
                    izTs = iztsp.tile([P, SC], f32, tag="izTs")
                    nc.scalar.copy(izTs[:], izT[:])
                    hT = htp.tile([P, SC], f32, tag="hT")
                    nc.vector.tensor_tensor_scan(hT[:], fT[:], izTs[:],
                                                 carries[g][:, 0:1],
                                                 op0=mult, op1=add)
                    nc.scalar.copy(carries[g][:, 0:1], hT[:, SC - 1:SC])
                    htc = psC.tile([P, NJ, P], f32, tag="htc")
                    for j in range(NJ):
                        nc.tensor.transpose(htc[:, j, :],
                                            hT[:, j * P:(j + 1) * P], ident[:])
                    nc.scalar.copy(hasm[:, :, gs], htc[:])

                for j in range(NJ):
                    t0 = s * SC + j * P
                    nc.sync.dma_start(h_d[t0:t0 + P, :], hasm[:, j, :])

    nc.compile()
    return nc


def _get_nc():
    if "nc" not in _CACHE:
        _CACHE["nc"] = _build_nc()
    return _CACHE["nc"]


def make_in_maps(f, z, i, hidden_init):
    f = np.asarray(f, dtype=np.float32)
    z = np.asarray(z, dtype=np.float32)
    i = np.asarray(i, dtype=np.float32)
    hidden_init = np.asarray(hidden_init, dtype=np.float32)
    in_maps = []
    for c in range(NCORES):
        b0 = c * BS
        in_maps.append({
            "f": np.ascontiguousarray(f[:, b0:b0 + BS, :]).reshape(T, C),
            "i": np.ascontiguousarray(i[:, b0:b0 + BS, :]).reshape(T, C),
            "z": np.ascontiguousarray(z[:, b0:b0 + BS, :]).reshape(T, C),
            "h0": np.ascontiguousarray(hidden_init[b0:b0 + BS, :]).reshape(C, 1),
        })
    return in_maps


def kernel(f, z, i, hidden_init):
    from concourse.bass_utils import run_bass_kernel_spmd

    in_maps = make_in_maps(f, z, i, hidden_init)
    res = run_bass_kernel_spmd(_get_nc(), in_maps, list(range(NCORES))).results
    out = np.empty((T, B, H), np.float32)
    for c in range(NCORES):
        out[:, c * BS:(c + 1) * BS, :] = res[c]["h"].reshape(T, BS, H)
    return out


# revision 7
# speedup vs baseline: 54.4619x; 54.4619x over previous
"""QRNN forget-mult recurrence h_t = i_t*z_t + f_t*h_{t-1} on 8 NeuronCores.

Sharding: batch dim B=32 split 4-per-core (data parallel). Per core the
[T=4096, B=4, H=256] slice is viewed as [T, C=1024] channels. The recurrence
runs on the DVE TensorTensorScan instruction, which scans along the free
dimension — so f and iz are PE-transposed into [channel, time] layout in
512-step chunks, scanned with a carried initial value, and transposed back.
"""

import numpy as np

T = 4096
B = 32
H = 256
NCORES = 8
BS = B // NCORES          # batches per core
C = BS * H                # channels per core
P = 128                   # partitions
NG = C // P               # channel groups per core
SC = 512                  # timesteps per scan chunk (one PSUM bank of f32)
NS = T // SC
NJ = SC // P              # 128-row transpose tiles per chunk

_CACHE = {}


def _build_nc(ins_bufs=8, iz_bufs=6, izts_bufs=4, ht_bufs=6, asm_bufs=2,
              out_engine="gpsimd", carry_direct=False, psA_bufs=3, psB_bufs=3,
              psC_bufs=2):
    import concourse.tile as tile
    from concourse import bacc, mybir
    from concourse.masks import make_identity

    f32 = mybir.dt.float32
    mult = mybir.AluOpType.mult
    add = mybir.AluOpType.add

    nc = bacc.Bacc("TRN2", target_bir_lowering=False, debug=False)
    f_d = nc.dram_tensor("f", [T, C], f32, kind="ExternalInput")
    i_d = nc.dram_tensor("i", [T, C], f32, kind="ExternalInput")
    z_d = nc.dram_tensor("z", [T, C], f32, kind="ExternalInput")
    h0_d = nc.dram_tensor("h0", [C, 1], f32, kind="ExternalInput")
    h_d = nc.dram_tensor("h", [T, C], f32, kind="ExternalOutput")

    with tile.TileContext(nc) as tc:
        with (
            tc.tile_pool(name="const", bufs=1) as constp,
            tc.tile_pool(name="ins", bufs=ins_bufs) as insp,
            tc.tile_pool(name="izp", bufs=iz_bufs) as izp,
            tc.tile_pool(name="izts", bufs=izts_bufs) as iztsp,
            tc.tile_pool(name="hts", bufs=ht_bufs) as htp,
            tc.tile_pool(name="asm", bufs=asm_bufs) as asmp,
            tc.tile_pool(name="psA", bufs=psA_bufs, space="PSUM") as psA,
            tc.tile_pool(name="psB", bufs=psB_bufs, space="PSUM") as psB,
            tc.tile_pool(name="psC", bufs=psC_bufs, space="PSUM") as psC,
        ):
            ident = constp.tile([P, P], f32)
            make_identity(nc, ident[:])

            carries = []
            for g in range(NG):
                cg = constp.tile([P, 1], f32, tag=f"carry{g}")
                nc.sync.dma_start(cg[:], h0_d[g * P:(g + 1) * P, :])
                carries.append(cg)
            # with carry_direct, the scan initial reads the previous chunk's
            # hT tile in place instead of a copied-out carry column
            prev_hT = [None] * NG

            for s in range(NS):
                fts, izts = [], []
                for j in range(NJ):
                    t0 = s * SC + j * P
                    ft = insp.tile([P, C], f32, tag="f")
                    nc.sync.dma_start(ft[:], f_d[t0:t0 + P, :])
                    it = insp.tile([P, C], f32, tag="i")
                    nc.sync.dma_start(it[:], i_d[t0:t0 + P, :])
                    zt = insp.tile([P, C], f32, tag="z")
                    nc.sync.dma_start(zt[:], z_d[t0:t0 + P, :])
                    izt = izp.tile([P, C], f32, tag="iz")
                    nc.vector.tensor_mul(izt[:], it[:], zt[:])
                    fts.append(ft)
                    izts.append(izt)

                hasm = asmp.tile([P, NJ, C], f32, tag="hasm")
                for g in range(NG):
                    gs = slice(g * P, (g + 1) * P)
                    fT = psA.tile([P, SC], f32, tag="fT")
                    izT = psB.tile([P, SC], f32, tag="izT")
                    for j in range(NJ):
                        nc.tensor.transpose(fT[:, j * P:(j + 1) * P],
                                            fts[j][:, gs], ident[:])
                        nc.tensor.transpose(izT[:, j * P:(j + 1) * P],
                                            izts[j][:, gs], ident[:])
                    # scan operands cannot both live in PSUM
                    izTs = iztsp.tile([P, SC], f32, tag="izTs")
                    nc.scalar.copy(izTs[:], izT[:])
                    hT = htp.tile([P, SC], f32, tag="hT")
                    if carry_direct:
                        init = (carries[g][:, 0:1] if prev_hT[g] is None
                                else prev_hT[g][:, SC - 1:SC])
                        prev_hT[g] = hT
                    else:
                        init = carries[g][:, 0:1]
                    nc.vector.tensor_tensor_scan(hT[:], fT[:], izTs[:], init,
                                                 op0=mult, op1=add)
                    if not carry_direct:
                        nc.scalar.copy(carries[g][:, 0:1], hT[:, SC - 1:SC])
                    htc = psC.tile([P, NJ, P], f32, tag="htc")
                    for j in range(NJ):
                        nc.tensor.transpose(htc[:, j, :],
                                            hT[:, j * P:(j + 1) * P], ident[:])
                    nc.scalar.copy(hasm[:, :, gs], htc[:])

                out_eng = getattr(nc, out_engine)
                for j in range(NJ):
                    t0 = s * SC + j * P
                    out_eng.dma_start(h_d[t0:t0 + P, :], hasm[:, j, :])

    nc.compile()
    return nc


def _get_nc():
    if "nc" not in _CACHE:
        _CACHE["nc"] = _build_nc()
    return _CACHE["nc"]


def make_in_maps(f, z, i, hidden_init):
    f = np.asarray(f, dtype=np.float32)
    z = np.asarray(z, dtype=np.float32)
    i = np.asarray(i, dtype=np.float32)
    hidden_init = np.asarray(hidden_init, dtype=np.float32)
    in_maps = []
    for c in range(NCORES):
        b0 = c * BS
        in_maps.append({
            "f": np.ascontiguousarray(f[:, b0:b0 + BS, :]).reshape(T, C),
            "i": np.ascontiguousarray(i[:, b0:b0 + BS, :]).reshape(T, C),
            "z": np.ascontiguousarray(z[:, b0:b0 + BS, :]).reshape(T, C),
            "h0": np.ascontiguousarray(hidden_init[b0:b0 + BS, :]).reshape(C, 1),
        })
    return in_maps


def kernel(f, z, i, hidden_init):
    import time

    from concourse.bass_utils import run_bass_kernel_spmd

    in_maps = make_in_maps(f, z, i, hidden_init)
    last_err = None
    for attempt in range(3):
        try:
            res = run_bass_kernel_spmd(
                _get_nc(), in_maps, list(range(NCORES))
            ).results
            break
        except Exception as e:  # transient device-unrecoverable states
            last_err = e
            time.sleep(2.0 * (attempt + 1))
    else:
        raise last_err
    out = np.empty((T, B, H), np.float32)
    for c in range(NCORES):
        out[:, c * BS:(c + 1) * BS, :] = res[c]["h"].reshape(T, BS, H)
    return out
